# revision 21
# baseline (speedup 1.0000x reference)
"""Trainium2 Bass kernel for a binarized (1w1a) BasicBlock:

    out = relu(bn2(conv2(sign(pad(relu(bn1(conv1(sign(pad(x)), sign(w1)))))), sign(w2))) + x)

with 2x3 convs, C=256, B=64, H=W=32, pad = (W: 1 left/right, H: 1 bottom).

Strategy: data-parallel over batch across 8 NeuronCores (8 images/core).
Per core the conv is an implicit GEMM: channels on partitions, each of the
6 kernel taps is a [K=128]x[M=128]x[N=512] matmul accumulated in PSUM over
(2 K-tiles x 6 taps). Inputs are binarized to bf16 (+-1/0 exact), so matmul
accumulation in fp32 PSUM is exact integer arithmetic. BN is folded on host
into per-channel scale/bias; conv1's bn+relu+sign epilogue collapses into a
single DVE tensor_scalar ((psum*inv1) is_gt (-bias1) -> {0,1}); conv2's
epilogue is scalar_tensor_tensor (psum*inv2 + x) followed by a Relu
activation with per-channel bias.
"""

import numpy as np
import ml_dtypes

import concourse.mybir as mybir
import concourse.tile as tile
from concourse import bacc
from concourse.bass_utils import run_bass_kernel_spmd

N_CORES = 8
B, C, H, W = 64, 256, 32, 32
BL = B // N_CORES          # images per core
P = 128
KT = C // P                # channel tiles (contraction / output)
HP, WP = H + 1, W + 2      # padded spatial dims (33, 34)
IMG = HP * WP              # 1122
NPOS = 6                   # 2x3 kernel taps
EPS = 1e-5

F32 = mybir.dt.float32
BF16 = mybir.dt.bfloat16
FP8 = mybir.dt.float8e4

# fp8 DoubleRow variant: shared-pad plane layout. Each padded row is 33 wide
# (32 data + 1 shared zero column that serves as row h's right pad AND row
# h+1's left pad), plus one leading zero and a zero bottom row. Conv output
# (h, w) lands at flat position h*33 + w of the 363-column PSUM chunks.
PITCH = 33
DATA0 = 1                   # leading zero (left pad of row 0)
PLANE = DATA0 + PITCH * PITCH   # 1090 = data extent incl bottom pad row
NJ = 3                      # chunks per image (11 rows each)
CH = 11 * PITCH             # 363
NPAD = 1168                 # >= 2*CH + max tap offset (67) + CH, mult of 16
ROWS_J = (11, 11, 10)       # valid output rows per chunk

VARIANT = "fp8"             # "bf16" | "fp8"

_CACHE = {}


def _build():
    return _build_fp8()


def _build_fp8():
    """fp8e4 DoubleRow variant: both channel tiles contract in one PE pass.

    Activations live as [128, 2, NPAD] fp8 tiles (ko-interleaved padded
    planes); each conv output chunk is a [128, 374] PSUM tile covering 11
    padded rows of one image, accumulated over the 6 taps with one
    DoubleRow matmul per tap.
    """
    if "nc" in _CACHE:
        return _CACHE["nc"]

    nc = bacc.Bacc("TRN2", target_bir_lowering=False, debug=False)

    x_d = nc.dram_tensor("x", [BL, C, H, W], F32, kind="ExternalInput")
    w1_d = nc.dram_tensor("w1t", [P, KT, NPOS, C], FP8, kind="ExternalInput")
    w2_d = nc.dram_tensor("w2t", [P, KT, NPOS, C], FP8, kind="ExternalInput")
    bnv_d = nc.dram_tensor("bnv", [4, C], F32, kind="ExternalInput")
    out_d = nc.dram_tensor("out", [BL, C, H, W], F32, kind="ExternalOutput")

    with tile.TileContext(nc) as tc:
        with (
            tc.tile_pool(name="res", bufs=1) as res,
            tc.tile_pool(name="tmp", bufs=4) as tmp,
            tc.tile_pool(name="stg", bufs=4) as stg,
            tc.tile_pool(name="ps", bufs=6, space="PSUM") as ps,
        ):
            xf32 = [[None] * BL for _ in range(KT)]
            xq1 = [None] * BL
            xq2 = [None] * BL

            def pad_memsets(q, eng):
                """Zero only the pad cells: leading zero, shared pad column,
                bottom pad row, tail. Small strided memsets, pinned off the
                Scalar engine so SIGNs aren't delayed."""
                v = q[:, :, DATA0:DATA0 + PITCH * PITCH].rearrange(
                    "c k (h w) -> c k h w", w=PITCH)
                eng.memset(q[:, :, 0:DATA0], 0.0)
                eng.memset(v[:, :, :, W:PITCH], 0.0)
                eng.memset(v[:, :, H:PITCH, 0:W], 0.0)
                eng.memset(q[:, :, PLANE:NPAD], 0.0)

            def interior(q, kt):
                return q[:, kt, DATA0:DATA0 + H * PITCH].rearrange(
                    "c (h w) -> c h w", w=PITCH)[:, :, 0:W]

            def load_b(b):
                # Two input DMA queues in parallel: kt=0 slice on the sync
                # HWDGE queue, kt=1 slice on the gpsimd queue.
                q1 = res.tile([P, KT, NPAD], FP8, tag=f"xq1_{b}", name=f"xq1_{b}")
                pad_memsets(q1, nc.gpsimd)
                xq1[b] = q1
                q2 = res.tile([P, KT, NPAD], FP8, tag=f"xq2_{b}", name=f"xq2_{b}")
                pad_memsets(q2, nc.vector)
                xq2[b] = q2
                for kt in range(KT):
                    xt = res.tile([P, H * W], F32, tag=f"xf_{kt}_{b}", name=f"xf_{kt}_{b}")
                    (nc.sync if kt == 0 else nc.gpsimd).dma_start(
                        xt[:],
                        x_d.ap()[b, kt * P:(kt + 1) * P].rearrange("c h w -> c (h w)"),
                    )
                    xf32[kt][b] = xt
                    nc.scalar.sign(
                        interior(q1, kt),
                        xt.rearrange("c (h w) -> c h w", w=W),
                    )

            # weights first on the otherwise-idle scalar queue
            w1sb = res.tile([P, KT, NPOS, C], FP8, tag="w1q", name="w1q")
            nc.scalar.dma_start(w1sb[:], w1_d.ap())
            bnsb = res.tile([P, 4 * KT], F32, tag="bnv", name="bnv")
            nc.scalar.dma_start(bnsb[:], bnv_d.ap().rearrange("v (t p) -> p (v t)", p=P))
            w2sb = res.tile([P, KT, NPOS, C], FP8, tag="w2q", name="w2q")
            nc.scalar.dma_start(w2sb[:], w2_d.ap())

            inv1sb = bnsb[:, 0 * KT:1 * KT]
            nb1sb = bnsb[:, 1 * KT:2 * KT]
            inv2sb = bnsb[:, 2 * KT:3 * KT]
            b2sb = bnsb[:, 3 * KT:4 * KT]

            for b in range(BL):
                load_b(b)

            def conv_groups(b, mt, wsb, src):
                """6-tap DoubleRow accumulation for the NJ chunks of (b, mt).

                pos-outer / chunk-inner so consecutive matmuls share lhsT.
                Returns the NJ psum tiles.
                """
                pts = [
                    ps.tile([P, CH], F32, tag="ps", name=f"ps_{b}_{mt}_{j}")
                    for j in range(NJ)
                ]
                for pos in range(NPOS):
                    kh, kw = divmod(pos, 3)
                    off = kh * PITCH + kw
                    for j in range(NJ):
                        nc.tensor.matmul(
                            pts[j][:],
                            wsb[:, :, pos, mt * P:(mt + 1) * P],
                            src[:, :, off + j * CH: off + j * CH + CH],
                            start=(pos == 0),
                            stop=(pos == NPOS - 1),
                            perf_mode=mybir.MatmulPerfMode.DoubleRow,
                        )
                return pts

            # ---- conv1 + binarize epilogue ----
            for b in range(BL):
                for mt in range(KT):
                    pts = conv_groups(b, mt, w1sb, xq1[b])
                    q2v = interior(xq2[b], mt)
                    for j in range(NJ):
                        r = ROWS_J[j]
                        nc.vector.tensor_scalar(
                            q2v[:, 11 * j:11 * j + r, :],
                            pts[j].rearrange("c (r w) -> c r w", w=PITCH)[:, 0:r, 0:W],
                            inv1sb[:, mt:mt + 1],
                            nb1sb[:, mt:mt + 1],
                            mybir.AluOpType.mult,
                            mybir.AluOpType.is_gt,
                        )

            # ---- conv2 + bn2 + residual + relu ----
            OUTQ = (nc.sync, nc.gpsimd, nc.scalar)
            for b in range(BL):
                for mt in range(KT):
                    pts = conv_groups(b, mt, w2sb, xq2[b])
                    ot = stg.tile([P, H * W], F32, tag="ot", name=f"ot_{b}_{mt}")
                    for j in range(NJ):
                        r = ROWS_J[j]
                        n = r * W
                        n0 = 11 * j * W
                        tt = tmp.tile([P, 11 * W], F32, tag="t2", name=f"t2_{b}_{mt}_{j}")
                        nc.vector.scalar_tensor_tensor(
                            tt[:, 0:n].rearrange("c (r w) -> c r w", w=W),
                            pts[j].rearrange("c (r w) -> c r w", w=PITCH)[:, 0:r, 0:W],
                            inv2sb[:, mt:mt + 1],
                            xf32[mt][b][:, n0:n0 + n].rearrange("c (r w) -> c r w", w=W),
                            mybir.AluOpType.mult,
                            mybir.AluOpType.add,
                        )
                        nc.scalar.activation(
                            ot[:, n0:n0 + n], tt[:, 0:n],
                            mybir.ActivationFunctionType.Relu,
                            bias=b2sb[:, mt:mt + 1],
                            scale=1.0,
                        )
                    OUTQ[(b * KT + mt) % 3].dma_start(
                        out_d.ap()[b, mt * P:(mt + 1) * P].rearrange("c h w -> c (h w)"),
                        ot[:],
                    )

    nc.compile()
    _CACHE["nc"] = nc
    return nc


def _prep(w1, w2, gamma1, beta1, mean1, var1, gamma2, beta2, mean2, var2):
    """Host-side: fold BN, binarize + lay out weights as lhsT tiles."""
    def fold(gamma, beta, mean, var):
        inv = (gamma.astype(np.float64) / np.sqrt(var.astype(np.float64) + EPS))
        inv = inv.astype(np.float32)
        bias = (beta.astype(np.float32) - mean.astype(np.float32) * inv)
        return inv, bias

    inv1, bias1 = fold(gamma1, beta1, mean1, var1)
    inv2, bias2 = fold(gamma2, beta2, mean2, var2)

    if VARIANT == "fp8":
        def wt(w):
            # [O, I, 2, 3] -> DoubleRow lhsT layout [ci, ko, pos, co]
            s = np.sign(w).astype(np.float32)
            arr = s.transpose(1, 2, 3, 0).reshape(KT, P, NPOS, C).transpose(1, 0, 2, 3)
            return np.ascontiguousarray(arr).astype(mybir.dt.np(FP8))
    else:
        def wt(w):
            # [O, I, 2, 3] -> lhsT layout [kt, ci, pos, co]
            s = np.sign(w).astype(ml_dtypes.bfloat16)
            return np.ascontiguousarray(
                s.transpose(1, 2, 3, 0).reshape(KT, P, NPOS, C)
            )

    bnv = np.ascontiguousarray(np.stack([inv1, -bias1, inv2, bias2]))
    return wt(w1), wt(w2), bnv


def kernel(x, w1, gamma1, beta1, mean1, var1,
           w2, gamma2, beta2, mean2, var2):
    x = np.asarray(x, dtype=np.float32)
    w1t, w2t, bnv = _prep(
        np.asarray(w1), np.asarray(w2),
        np.asarray(gamma1), np.asarray(beta1), np.asarray(mean1), np.asarray(var1),
        np.asarray(gamma2), np.asarray(beta2), np.asarray(mean2), np.asarray(var2),
    )

    nc = _build()
    in_maps = []
    for c in range(N_CORES):
        in_maps.append({
            "x": np.ascontiguousarray(x[c * BL:(c + 1) * BL]),
            "w1t": w1t, "w2t": w2t, "bnv": bnv,
        })

    res = run_bass_kernel_spmd(nc, in_maps, core_ids=list(range(N_CORES)))
    out = np.concatenate([r["out"] for r in res.results], axis=0)
    return out


# revision 26
# speedup vs baseline: 1.0352x; 1.0352x over previous
"""Trainium2 Bass kernel for a binarized (1w1a) BasicBlock:

    out = relu(bn2(conv2(sign(pad(relu(bn1(conv1(sign(pad(x)), sign(w1)))))), sign(w2))) + x)

with 2x3 convs, C=256, B=64, H=W=32, pad = (W: 1 left/right, H: 1 bottom).

Strategy: data-parallel over batch across 8 NeuronCores (8 images/core).
Per core the conv is an implicit GEMM: channels on partitions, each of the
6 kernel taps is a [K=128]x[M=128]x[N=512] matmul accumulated in PSUM over
(2 K-tiles x 6 taps). Inputs are binarized to bf16 (+-1/0 exact), so matmul
accumulation in fp32 PSUM is exact integer arithmetic. BN is folded on host
into per-channel scale/bias; conv1's bn+relu+sign epilogue collapses into a
single DVE tensor_scalar ((psum*inv1) is_gt (-bias1) -> {0,1}); conv2's
epilogue is scalar_tensor_tensor (psum*inv2 + x) followed by a Relu
activation with per-channel bias.
"""

import numpy as np
import ml_dtypes

import concourse.mybir as mybir
import concourse.tile as tile
from concourse import bacc
from concourse.bass_utils import run_bass_kernel_spmd

N_CORES = 8
B, C, H, W = 64, 256, 32, 32
BL = B // N_CORES          # images per core
P = 128
KT = C // P                # channel tiles (contraction / output)
HP, WP = H + 1, W + 2      # padded spatial dims (33, 34)
IMG = HP * WP              # 1122
NPOS = 6                   # 2x3 kernel taps
EPS = 1e-5

F32 = mybir.dt.float32
BF16 = mybir.dt.bfloat16
FP8 = mybir.dt.float8e4

# fp8 DoubleRow variant: shared-pad plane layout. Each padded row is 33 wide
# (32 data + 1 shared zero column that serves as row h's right pad AND row
# h+1's left pad), plus one leading zero and a zero bottom row. Conv output
# (h, w) lands at flat position h*33 + w of the 363-column PSUM chunks.
PITCH = 33
DATA0 = 1                   # leading zero (left pad of row 0)
PLANE = DATA0 + PITCH * PITCH   # 1090 = data extent incl bottom pad row
NJ = 3                      # chunks per image (11 rows each)
CH = 11 * PITCH             # 363
NPAD = 1168                 # >= 2*CH + max tap offset (67) + CH, mult of 16
ROWS_J = (11, 11, 10)       # valid output rows per chunk

VARIANT = "fp8"             # "bf16" | "fp8"

_CACHE = {}


def _build():
    return _build_fp8()


def _build_fp8():
    """fp8e4 DoubleRow variant: both channel tiles contract in one PE pass.

    Activations live as [128, 2, NPAD] fp8 tiles (ko-interleaved padded
    planes); each conv output chunk is a [128, 374] PSUM tile covering 11
    padded rows of one image, accumulated over the 6 taps with one
    DoubleRow matmul per tap.
    """
    if "nc" in _CACHE:
        return _CACHE["nc"]

    nc = bacc.Bacc("TRN2", target_bir_lowering=False, debug=False)

    # x arrives host-transposed to [p, b, kt, hw] so each SBUF partition's
    # data is one long contiguous DRAM run (8KB per image, 48KB for the
    # b>=2 bulk) -- the DMA engines are descriptor-rate bound on 4KB lines.
    x_d = nc.dram_tensor("x", [P, BL, KT, H * W], F32, kind="ExternalInput")
    w1_d = nc.dram_tensor("w1t", [P, KT, NPOS, C], FP8, kind="ExternalInput")
    w2_d = nc.dram_tensor("w2t", [P, KT, NPOS, C], FP8, kind="ExternalInput")
    bnv_d = nc.dram_tensor("bnv", [4, C], F32, kind="ExternalInput")
    out_d = nc.dram_tensor("out", [BL, C, H, W], F32, kind="ExternalOutput")

    with tile.TileContext(nc) as tc:
        with (
            tc.tile_pool(name="res", bufs=1) as res,
            tc.tile_pool(name="tmp", bufs=4) as tmp,
            tc.tile_pool(name="stg", bufs=4) as stg,
            tc.tile_pool(name="ps", bufs=6, space="PSUM") as ps,
        ):
            xq1 = [None] * BL
            xq2 = [None] * BL

            def pad_memsets(q, eng):
                """Zero only the pad cells: leading zero, shared pad column,
                bottom pad row, tail. Small strided memsets, pinned off the
                Scalar engine so SIGNs aren't delayed."""
                v = q[:, :, DATA0:DATA0 + PITCH * PITCH].rearrange(
                    "c k (h w) -> c k h w", w=PITCH)
                eng.memset(q[:, :, 0:DATA0], 0.0)
                eng.memset(v[:, :, :, W:PITCH], 0.0)
                eng.memset(v[:, :, H:PITCH, 0:W], 0.0)
                eng.memset(q[:, :, PLANE:NPAD], 0.0)

            def interior(q, kt):
                return q[:, kt, DATA0:DATA0 + H * PITCH].rearrange(
                    "c (h w) -> c h w", w=PITCH)[:, :, 0:W]

            # Input staging: three tiles (b=0, b=1, bulk b=2..7), each one
            # big-line DMA on the sync queue. xf(b) -> [128, 1024] f32 view.
            xg = [None] * 3
            xg[0] = res.tile([P, 1, KT, H * W], F32, tag="xg0", name="xg0")
            nc.sync.dma_start(xg[0][:], x_d.ap()[:, 0:1])
            xg[1] = res.tile([P, 1, KT, H * W], F32, tag="xg1", name="xg1")
            nc.sync.dma_start(xg[1][:], x_d.ap()[:, 1:2])
            xg[2] = res.tile([P, BL - 2, KT, H * W], F32, tag="xg2", name="xg2")
            nc.sync.dma_start(xg[2][:], x_d.ap()[:, 2:BL])

            def xf(kt, b):
                g = xg[b] if b < 2 else xg[2]
                return g[:, 0 if b < 2 else b - 2, kt]

            # weights + BN on the scalar engine's queue, in parallel
            w1sb = res.tile([P, KT, NPOS, C], FP8, tag="w1q", name="w1q")
            nc.scalar.dma_start(w1sb[:], w1_d.ap())
            bnsb = res.tile([P, 4 * KT], F32, tag="bnv", name="bnv")
            nc.scalar.dma_start(bnsb[:], bnv_d.ap().rearrange("v (t p) -> p (v t)", p=P))
            w2sb = res.tile([P, KT, NPOS, C], FP8, tag="w2q", name="w2q")
            nc.scalar.dma_start(w2sb[:], w2_d.ap())

            inv1sb = bnsb[:, 0 * KT:1 * KT]
            nb1sb = bnsb[:, 1 * KT:2 * KT]
            inv2sb = bnsb[:, 2 * KT:3 * KT]
            b2sb = bnsb[:, 3 * KT:4 * KT]

            for b in range(BL):
                q1 = res.tile([P, KT, NPAD], FP8, tag=f"xq1_{b}", name=f"xq1_{b}")
                pad_memsets(q1, nc.gpsimd)
                xq1[b] = q1
                q2 = res.tile([P, KT, NPAD], FP8, tag=f"xq2_{b}", name=f"xq2_{b}")
                pad_memsets(q2, nc.vector)
                xq2[b] = q2
                for kt in range(KT):
                    nc.scalar.sign(
                        interior(q1, kt),
                        xf(kt, b).rearrange("c (h w) -> c h w", w=W),
                    )

            def conv_groups(b, mt, wsb, src):
                """6-tap DoubleRow accumulation for the NJ chunks of (b, mt).

                pos-outer / chunk-inner so consecutive matmuls share lhsT.
                Returns the NJ psum tiles.
                """
                pts = [
                    ps.tile([P, CH], F32, tag="ps", name=f"ps_{b}_{mt}_{j}")
                    for j in range(NJ)
                ]
                for pos in range(NPOS):
                    kh, kw = divmod(pos, 3)
                    off = kh * PITCH + kw
                    for j in range(NJ):
                        nc.tensor.matmul(
                            pts[j][:],
                            wsb[:, :, pos, mt * P:(mt + 1) * P],
                            src[:, :, off + j * CH: off + j * CH + CH],
                            start=(pos == 0),
                            stop=(pos == NPOS - 1),
                            perf_mode=mybir.MatmulPerfMode.DoubleRow,
                        )
                return pts

            # ---- conv1 + binarize epilogue ----
            for b in range(BL):
                for mt in range(KT):
                    pts = conv_groups(b, mt, w1sb, xq1[b])
                    q2v = interior(xq2[b], mt)
                    for j in range(NJ):
                        r = ROWS_J[j]
                        nc.vector.tensor_scalar(
                            q2v[:, 11 * j:11 * j + r, :],
                            pts[j].rearrange("c (r w) -> c r w", w=PITCH)[:, 0:r, 0:W],
                            inv1sb[:, mt:mt + 1],
                            nb1sb[:, mt:mt + 1],
                            mybir.AluOpType.mult,
                            mybir.AluOpType.is_gt,
                        )

            # ---- conv2 + bn2 + residual + relu ----
            OUTQ = (nc.sync, nc.gpsimd, nc.scalar)
            for b in range(BL):
                for mt in range(KT):
                    pts = conv_groups(b, mt, w2sb, xq2[b])
                    ot = stg.tile([P, H * W], F32, tag="ot", name=f"ot_{b}_{mt}")
                    for j in range(NJ):
                        r = ROWS_J[j]
                        n = r * W
                        n0 = 11 * j * W
                        tt = tmp.tile([P, 11 * W], F32, tag="t2", name=f"t2_{b}_{mt}_{j}")
                        nc.vector.scalar_tensor_tensor(
                            tt[:, 0:n].rearrange("c (r w) -> c r w", w=W),
                            pts[j].rearrange("c (r w) -> c r w", w=PITCH)[:, 0:r, 0:W],
                            inv2sb[:, mt:mt + 1],
                            xf(mt, b)[:, n0:n0 + n].rearrange("c (r w) -> c r w", w=W),
                            mybir.AluOpType.mult,
                            mybir.AluOpType.add,
                        )
                        nc.scalar.activation(
                            ot[:, n0:n0 + n], tt[:, 0:n],
                            mybir.ActivationFunctionType.Relu,
                            bias=b2sb[:, mt:mt + 1],
                            scale=1.0,
                        )
                    OUTQ[(b * KT + mt) % 3].dma_start(
                        out_d.ap()[b, mt * P:(mt + 1) * P].rearrange("c h w -> c (h w)"),
                        ot[:],
                    )

    nc.compile()
    _CACHE["nc"] = nc
    return nc


def _prep(w1, w2, gamma1, beta1, mean1, var1, gamma2, beta2, mean2, var2):
    """Host-side: fold BN, binarize + lay out weights as lhsT tiles."""
    def fold(gamma, beta, mean, var):
        inv = (gamma.astype(np.float64) / np.sqrt(var.astype(np.float64) + EPS))
        inv = inv.astype(np.float32)
        bias = (beta.astype(np.float32) - mean.astype(np.float32) * inv)
        return inv, bias

    inv1, bias1 = fold(gamma1, beta1, mean1, var1)
    inv2, bias2 = fold(gamma2, beta2, mean2, var2)

    if VARIANT == "fp8":
        def wt(w):
            # [O, I, 2, 3] -> DoubleRow lhsT layout [ci, ko, pos, co]
            s = np.sign(w).astype(np.float32)
            arr = s.transpose(1, 2, 3, 0).reshape(KT, P, NPOS, C).transpose(1, 0, 2, 3)
            return np.ascontiguousarray(arr).astype(mybir.dt.np(FP8))
    else:
        def wt(w):
            # [O, I, 2, 3] -> lhsT layout [kt, ci, pos, co]
            s = np.sign(w).astype(ml_dtypes.bfloat16)
            return np.ascontiguousarray(
                s.transpose(1, 2, 3, 0).reshape(KT, P, NPOS, C)
            )

    bnv = np.ascontiguousarray(np.stack([inv1, -bias1, inv2, bias2]))
    return wt(w1), wt(w2), bnv


def _in_maps(x, w1t, w2t, bnv):
    """Per-core input dicts; x is transposed to the [p, b, kt, hw] layout."""
    maps = []
    for c in range(N_CORES):
        xs = x[c * BL:(c + 1) * BL]                       # [BL, C, H, W]
        xh = np.ascontiguousarray(
            xs.reshape(BL, KT, P, H * W).transpose(2, 0, 1, 3))
        maps.append({"x": xh, "w1t": w1t, "w2t": w2t, "bnv": bnv})
    return maps


def kernel(x, w1, gamma1, beta1, mean1, var1,
           w2, gamma2, beta2, mean2, var2):
    x = np.asarray(x, dtype=np.float32)
    w1t, w2t, bnv = _prep(
        np.asarray(w1), np.asarray(w2),
        np.asarray(gamma1), np.asarray(beta1), np.asarray(mean1), np.asarray(var1),
        np.asarray(gamma2), np.asarray(beta2), np.asarray(mean2), np.asarray(var2),
    )

    nc = _build()
    in_maps = _in_maps(x, w1t, w2t, bnv)

    res = run_bass_kernel_spmd(nc, in_maps, core_ids=list(range(N_CORES)))
    out = np.concatenate([r["out"] for r in res.results], axis=0)
    return out


# revision 27
# speedup vs baseline: 1.0521x; 1.0164x over previous
"""Trainium2 Bass kernel for a binarized (1w1a) BasicBlock:

    out = relu(bn2(conv2(sign(pad(relu(bn1(conv1(sign(pad(x)), sign(w1)))))), sign(w2))) + x)

with 2x3 convs, C=256, B=64, H=W=32, pad = (W: 1 left/right, H: 1 bottom).

Strategy: data-parallel over batch across 8 NeuronCores (8 images/core).
Per core the conv is an implicit GEMM: channels on partitions, each of the
6 kernel taps is a [K=128]x[M=128]x[N=512] matmul accumulated in PSUM over
(2 K-tiles x 6 taps). Inputs are binarized to bf16 (+-1/0 exact), so matmul
accumulation in fp32 PSUM is exact integer arithmetic. BN is folded on host
into per-channel scale/bias; conv1's bn+relu+sign epilogue collapses into a
single DVE tensor_scalar ((psum*inv1) is_gt (-bias1) -> {0,1}); conv2's
epilogue is scalar_tensor_tensor (psum*inv2 + x) followed by a Relu
activation with per-channel bias.
"""

import numpy as np
import ml_dtypes

import concourse.mybir as mybir
import concourse.tile as tile
from concourse import bacc
from concourse.bass_utils import run_bass_kernel_spmd

N_CORES = 8
B, C, H, W = 64, 256, 32, 32
BL = B // N_CORES          # images per core
P = 128
KT = C // P                # channel tiles (contraction / output)
HP, WP = H + 1, W + 2      # padded spatial dims (33, 34)
IMG = HP * WP              # 1122
NPOS = 6                   # 2x3 kernel taps
EPS = 1e-5

F32 = mybir.dt.float32
BF16 = mybir.dt.bfloat16
FP8 = mybir.dt.float8e4

# fp8 DoubleRow variant: shared-pad plane layout. Each padded row is 33 wide
# (32 data + 1 shared zero column that serves as row h's right pad AND row
# h+1's left pad), plus one leading zero and a zero bottom row. Conv output
# (h, w) lands at flat position h*33 + w of the 363-column PSUM chunks.
PITCH = 33
DATA0 = 1                   # leading zero (left pad of row 0)
PLANE = DATA0 + PITCH * PITCH   # 1090 = data extent incl bottom pad row
NJ = 3                      # chunks per image (11 rows each)
CH = 11 * PITCH             # 363
NPAD = 1168                 # >= 2*CH + max tap offset (67) + CH, mult of 16
ROWS_J = (11, 11, 10)       # valid output rows per chunk

VARIANT = "fp8"             # "bf16" | "fp8"

_CACHE = {}


def _build():
    return _build_fp8()


def _build_fp8():
    """fp8e4 DoubleRow variant: both channel tiles contract in one PE pass.

    Activations live as [128, 2, NPAD] fp8 tiles (ko-interleaved padded
    planes); each conv output chunk is a [128, 374] PSUM tile covering 11
    padded rows of one image, accumulated over the 6 taps with one
    DoubleRow matmul per tap.
    """
    if "nc" in _CACHE:
        return _CACHE["nc"]

    nc = bacc.Bacc("TRN2", target_bir_lowering=False, debug=False)

    # x arrives host-transposed to [p, b, kt, hw] so each SBUF partition's
    # data is one long contiguous DRAM run (8KB per image, 48KB for the
    # b>=2 bulk) -- the DMA engines are descriptor-rate bound on 4KB lines.
    x_d = nc.dram_tensor("x", [P, BL, KT, H * W], F32, kind="ExternalInput")
    w1_d = nc.dram_tensor("w1t", [P, KT, NPOS, C], FP8, kind="ExternalInput")
    w2_d = nc.dram_tensor("w2t", [P, KT, NPOS, C], FP8, kind="ExternalInput")
    bnv_d = nc.dram_tensor("bnv", [4, C], F32, kind="ExternalInput")
    out_d = nc.dram_tensor("out", [BL, C, H, W], F32, kind="ExternalOutput")

    with tile.TileContext(nc) as tc:
        with (
            tc.tile_pool(name="res", bufs=1) as res,
            tc.tile_pool(name="tmp", bufs=4) as tmp,
            tc.tile_pool(name="stg", bufs=4) as stg,
            tc.tile_pool(name="ps", bufs=6, space="PSUM") as ps,
        ):
            xq1 = [None] * BL
            xq2 = [None] * BL

            def pad_memsets(q, eng):
                """Zero only the pad cells: leading zero, shared pad column,
                bottom pad row, tail. Small strided memsets, pinned off the
                Scalar engine so SIGNs aren't delayed."""
                v = q[:, :, DATA0:DATA0 + PITCH * PITCH].rearrange(
                    "c k (h w) -> c k h w", w=PITCH)
                eng.memset(q[:, :, 0:DATA0], 0.0)
                eng.memset(v[:, :, :, W:PITCH], 0.0)
                eng.memset(v[:, :, H:PITCH, 0:W], 0.0)
                eng.memset(q[:, :, PLANE:NPAD], 0.0)

            def interior(q, kt):
                return q[:, kt, DATA0:DATA0 + H * PITCH].rearrange(
                    "c (h w) -> c h w", w=PITCH)[:, :, 0:W]

            # Input staging: one [P, KT, HW] tile per image, big-line DMAs
            # interleaved over the two HW queues (sync / scalar) so each
            # image lands well before the PE reaches it. Image 0 is split by
            # kt across both queues to cut the first-matmul latency.
            xg = [
                res.tile([P, KT, H * W], F32, tag=f"xg{b}", name=f"xg{b}")
                for b in range(BL)
            ]

            def xf(kt, b):
                return xg[b][:, kt]

            nc.sync.dma_start(xg[0][:, 0:1], x_d.ap()[:, 0, 0:1])
            w1sb = res.tile([P, KT, NPOS, C], FP8, tag="w1q", name="w1q")
            nc.scalar.dma_start(w1sb[:], w1_d.ap())
            nc.scalar.dma_start(xg[0][:, 1:2], x_d.ap()[:, 0, 1:2])
            nc.scalar.dma_start(xg[1][:], x_d.ap()[:, 1])
            nc.sync.dma_start(xg[2][:], x_d.ap()[:, 2])
            bnsb = res.tile([P, 4 * KT], F32, tag="bnv", name="bnv")
            nc.scalar.dma_start(bnsb[:], bnv_d.ap().rearrange("v (t p) -> p (v t)", p=P))
            for b in range(3, BL):
                (nc.scalar if b % 2 else nc.sync).dma_start(xg[b][:], x_d.ap()[:, b])
            w2sb = res.tile([P, KT, NPOS, C], FP8, tag="w2q", name="w2q")
            nc.scalar.dma_start(w2sb[:], w2_d.ap())

            inv1sb = bnsb[:, 0 * KT:1 * KT]
            nb1sb = bnsb[:, 1 * KT:2 * KT]
            inv2sb = bnsb[:, 2 * KT:3 * KT]
            b2sb = bnsb[:, 3 * KT:4 * KT]

            for b in range(BL):
                q1 = res.tile([P, KT, NPAD], FP8, tag=f"xq1_{b}", name=f"xq1_{b}")
                pad_memsets(q1, nc.gpsimd)
                xq1[b] = q1
                q2 = res.tile([P, KT, NPAD], FP8, tag=f"xq2_{b}", name=f"xq2_{b}")
                pad_memsets(q2, nc.vector)
                xq2[b] = q2
                for kt in range(KT):
                    nc.scalar.sign(
                        interior(q1, kt),
                        xf(kt, b).rearrange("c (h w) -> c h w", w=W),
                    )

            def conv_groups(b, mt, wsb, src):
                """6-tap DoubleRow accumulation for the NJ chunks of (b, mt).

                pos-outer / chunk-inner so consecutive matmuls share lhsT.
                Returns the NJ psum tiles.
                """
                pts = [
                    ps.tile([P, CH], F32, tag="ps", name=f"ps_{b}_{mt}_{j}")
                    for j in range(NJ)
                ]
                for pos in range(NPOS):
                    kh, kw = divmod(pos, 3)
                    off = kh * PITCH + kw
                    for j in range(NJ):
                        nc.tensor.matmul(
                            pts[j][:],
                            wsb[:, :, pos, mt * P:(mt + 1) * P],
                            src[:, :, off + j * CH: off + j * CH + CH],
                            start=(pos == 0),
                            stop=(pos == NPOS - 1),
                            perf_mode=mybir.MatmulPerfMode.DoubleRow,
                        )
                return pts

            # ---- conv1 + binarize epilogue ----
            for b in range(BL):
                for mt in range(KT):
                    pts = conv_groups(b, mt, w1sb, xq1[b])
                    q2v = interior(xq2[b], mt)
                    for j in range(NJ):
                        r = ROWS_J[j]
                        nc.vector.tensor_scalar(
                            q2v[:, 11 * j:11 * j + r, :],
                            pts[j].rearrange("c (r w) -> c r w", w=PITCH)[:, 0:r, 0:W],
                            inv1sb[:, mt:mt + 1],
                            nb1sb[:, mt:mt + 1],
                            mybir.AluOpType.mult,
                            mybir.AluOpType.is_gt,
                        )

            # ---- conv2 + bn2 + residual + relu ----
            OUTQ = (nc.sync, nc.gpsimd, nc.scalar)
            for b in range(BL):
                for mt in range(KT):
                    pts = conv_groups(b, mt, w2sb, xq2[b])
                    ot = stg.tile([P, H * W], F32, tag="ot", name=f"ot_{b}_{mt}")
                    for j in range(NJ):
                        r = ROWS_J[j]
                        n = r * W
                        n0 = 11 * j * W
                        tt = tmp.tile([P, 11 * W], F32, tag="t2", name=f"t2_{b}_{mt}_{j}")
                        nc.vector.scalar_tensor_tensor(
                            tt[:, 0:n].rearrange("c (r w) -> c r w", w=W),
                            pts[j].rearrange("c (r w) -> c r w", w=PITCH)[:, 0:r, 0:W],
                            inv2sb[:, mt:mt + 1],
                            xf(mt, b)[:, n0:n0 + n].rearrange("c (r w) -> c r w", w=W),
                            mybir.AluOpType.mult,
                            mybir.AluOpType.add,
                        )
                        nc.scalar.activation(
                            ot[:, n0:n0 + n], tt[:, 0:n],
                            mybir.ActivationFunctionType.Relu,
                            bias=b2sb[:, mt:mt + 1],
                            scale=1.0,
                        )
                    OUTQ[(b * KT + mt) % 3].dma_start(
                        out_d.ap()[b, mt * P:(mt + 1) * P].rearrange("c h w -> c (h w)"),
                        ot[:],
                    )

    nc.compile()
    _CACHE["nc"] = nc
    return nc


def _prep(w1, w2, gamma1, beta1, mean1, var1, gamma2, beta2, mean2, var2):
    """Host-side: fold BN, binarize + lay out weights as lhsT tiles."""
    def fold(gamma, beta, mean, var):
        inv = (gamma.astype(np.float64) / np.sqrt(var.astype(np.float64) + EPS))
        inv = inv.astype(np.float32)
        bias = (beta.astype(np.float32) - mean.astype(np.float32) * inv)
        return inv, bias

    inv1, bias1 = fold(gamma1, beta1, mean1, var1)
    inv2, bias2 = fold(gamma2, beta2, mean2, var2)

    if VARIANT == "fp8":
        def wt(w):
            # [O, I, 2, 3] -> DoubleRow lhsT layout [ci, ko, pos, co]
            s = np.sign(w).astype(np.float32)
            arr = s.transpose(1, 2, 3, 0).reshape(KT, P, NPOS, C).transpose(1, 0, 2, 3)
            return np.ascontiguousarray(arr).astype(mybir.dt.np(FP8))
    else:
        def wt(w):
            # [O, I, 2, 3] -> lhsT layout [kt, ci, pos, co]
            s = np.sign(w).astype(ml_dtypes.bfloat16)
            return np.ascontiguousarray(
                s.transpose(1, 2, 3, 0).reshape(KT, P, NPOS, C)
            )

    bnv = np.ascontiguousarray(np.stack([inv1, -bias1, inv2, bias2]))
    return wt(w1), wt(w2), bnv


def _in_maps(x, w1t, w2t, bnv):
    """Per-core input dicts; x is transposed to the [p, b, kt, hw] layout."""
    maps = []
    for c in range(N_CORES):
        xs = x[c * BL:(c + 1) * BL]                       # [BL, C, H, W]
        xh = np.ascontiguousarray(
            xs.reshape(BL, KT, P, H * W).transpose(2, 0, 1, 3))
        maps.append({"x": xh, "w1t": w1t, "w2t": w2t, "bnv": bnv})
    return maps


def kernel(x, w1, gamma1, beta1, mean1, var1,
           w2, gamma2, beta2, mean2, var2):
    x = np.asarray(x, dtype=np.float32)
    w1t, w2t, bnv = _prep(
        np.asarray(w1), np.asarray(w2),
        np.asarray(gamma1), np.asarray(beta1), np.asarray(mean1), np.asarray(var1),
        np.asarray(gamma2), np.asarray(beta2), np.asarray(mean2), np.asarray(var2),
    )

    nc = _build()
    in_maps = _in_maps(x, w1t, w2t, bnv)

    res = run_bass_kernel_spmd(nc, in_maps, core_ids=list(range(N_CORES)))
    out = np.concatenate([r["out"] for r in res.results], axis=0)
    return out


# revision 28
# speedup vs baseline: 1.1185x; 1.0630x over previous
"""Trainium2 Bass kernel for a binarized (1w1a) BasicBlock:

    out = relu(bn2(conv2(sign(pad(relu(bn1(conv1(sign(pad(x)), sign(w1)))))), sign(w2))) + x)

with 2x3 convs, C=256, B=64, H=W=32, pad = (W: 1 left/right, H: 1 bottom).

Strategy: data-parallel over batch across 8 NeuronCores (8 images/core).
Per core the conv is an implicit GEMM: channels on partitions, each of the
6 kernel taps is a [K=128]x[M=128]x[N=512] matmul accumulated in PSUM over
(2 K-tiles x 6 taps). Inputs are binarized to bf16 (+-1/0 exact), so matmul
accumulation in fp32 PSUM is exact integer arithmetic. BN is folded on host
into per-channel scale/bias; conv1's bn+relu+sign epilogue collapses into a
single DVE tensor_scalar ((psum*inv1) is_gt (-bias1) -> {0,1}); conv2's
epilogue is scalar_tensor_tensor (psum*inv2 + x) followed by a Relu
activation with per-channel bias.
"""

import numpy as np
import ml_dtypes

import concourse.mybir as mybir
import concourse.tile as tile
from concourse import bacc
from concourse.bass_utils import run_bass_kernel_spmd

N_CORES = 8
B, C, H, W = 64, 256, 32, 32
BL = B // N_CORES          # images per core
P = 128
KT = C // P                # channel tiles (contraction / output)
HP, WP = H + 1, W + 2      # padded spatial dims (33, 34)
IMG = HP * WP              # 1122
NPOS = 6                   # 2x3 kernel taps
EPS = 1e-5

F32 = mybir.dt.float32
BF16 = mybir.dt.bfloat16
FP8 = mybir.dt.float8e4

# fp8 DoubleRow variant: shared-pad plane layout. Each padded row is 33 wide
# (32 data + 1 shared zero column that serves as row h's right pad AND row
# h+1's left pad), plus one leading zero and a zero bottom row. Conv output
# (h, w) lands at flat position h*33 + w of the 363-column PSUM chunks.
PITCH = 33
DATA0 = 1                   # leading zero (left pad of row 0)
PLANE = DATA0 + PITCH * PITCH   # 1090 = data extent incl bottom pad row
NJ = 3                      # chunks per image (11 rows each)
CH = 11 * PITCH             # 363
NPAD = 1168                 # >= 2*CH + max tap offset (67) + CH, mult of 16
ROWS_J = (11, 11, 10)       # valid output rows per chunk

VARIANT = "fp8"             # "bf16" | "fp8"

_CACHE = {}


def _build():
    return _build_fp8()


def _build_fp8():
    """fp8e4 DoubleRow variant: both channel tiles contract in one PE pass.

    Activations live as [128, 2, NPAD] fp8 tiles (ko-interleaved padded
    planes); each conv output chunk is a [128, 374] PSUM tile covering 11
    padded rows of one image, accumulated over the 6 taps with one
    DoubleRow matmul per tap.
    """
    if "nc" in _CACHE:
        return _CACHE["nc"]

    nc = bacc.Bacc("TRN2", target_bir_lowering=False, debug=False)

    # x arrives host-transposed to [p, b, kt, hw] so each SBUF partition's
    # data is one long contiguous DRAM run (8KB per image, 48KB for the
    # b>=2 bulk) -- the DMA engines are descriptor-rate bound on 4KB lines.
    x_d = nc.dram_tensor("x", [P, BL, KT, H * W], F32, kind="ExternalInput")
    w1_d = nc.dram_tensor("w1t", [P, KT, NPOS, C], FP8, kind="ExternalInput")
    w2_d = nc.dram_tensor("w2t", [P, KT, NPOS, C], FP8, kind="ExternalInput")
    bnv_d = nc.dram_tensor("bnv", [4, C], F32, kind="ExternalInput")
    out_d = nc.dram_tensor("out", [BL, C, H, W], F32, kind="ExternalOutput")

    with tile.TileContext(nc) as tc:
        with (
            tc.tile_pool(name="res", bufs=1) as res,
            tc.tile_pool(name="tmp", bufs=4) as tmp,
            tc.tile_pool(name="stg", bufs=4) as stg,
            tc.tile_pool(name="ps", bufs=6, space="PSUM") as ps,
        ):
            xq1 = [None] * BL
            xq2 = [None] * BL

            def pad_memsets(q, eng):
                """Zero only the pad cells: leading zero, shared pad column,
                bottom pad row, tail. Small strided memsets, pinned off the
                Scalar engine so SIGNs aren't delayed."""
                v = q[:, :, DATA0:DATA0 + PITCH * PITCH].rearrange(
                    "c k (h w) -> c k h w", w=PITCH)
                eng.memset(q[:, :, 0:DATA0], 0.0)
                eng.memset(v[:, :, :, W:PITCH], 0.0)
                eng.memset(v[:, :, H:PITCH, 0:W], 0.0)
                eng.memset(q[:, :, PLANE:NPAD], 0.0)

            def interior(q, kt):
                return q[:, kt, DATA0:DATA0 + H * PITCH].rearrange(
                    "c (h w) -> c h w", w=PITCH)[:, :, 0:W]

            # Input staging: one [P, KT, HW] tile per image (b=0 split per kt
            # so each half only waits its own queue). Big-line DMAs alternate
            # between the sync and scalar HW queues; later images' DMA-issue
            # instructions are emitted between sign batches so the scalar
            # ENGINE isn't busy issuing when the first signs become ready.
            xg = [None] * BL
            xg0 = [
                res.tile([P, 1, H * W], F32, tag=f"xg0_{kt}", name=f"xg0_{kt}")
                for kt in range(KT)
            ]

            def xf(kt, b):
                if b == 0:
                    return xg0[kt][:, 0]
                return xg[b][:, kt]

            def x_dma(b):
                xg[b] = res.tile([P, KT, H * W], F32, tag=f"xg{b}", name=f"xg{b}")
                (nc.scalar if b % 2 else nc.sync).dma_start(
                    xg[b][:], x_d.ap()[:, b])

            def prep_b(b):
                q1 = res.tile([P, KT, NPAD], FP8, tag=f"xq1_{b}", name=f"xq1_{b}")
                pad_memsets(q1, nc.gpsimd)
                xq1[b] = q1
                q2 = res.tile([P, KT, NPAD], FP8, tag=f"xq2_{b}", name=f"xq2_{b}")
                pad_memsets(q2, nc.vector)
                xq2[b] = q2
                for kt in range(KT):
                    nc.scalar.sign(
                        interior(q1, kt),
                        xf(kt, b).rearrange("c (h w) -> c h w", w=W),
                    )

            nc.sync.dma_start(xg0[0][:], x_d.ap()[:, 0, 0:1])
            nc.scalar.dma_start(xg0[1][:], x_d.ap()[:, 0, 1:2])
            w1sb = res.tile([P, KT, NPOS, C], FP8, tag="w1q", name="w1q")
            nc.scalar.dma_start(w1sb[:], w1_d.ap())
            x_dma(1)
            x_dma(2)
            prep_b(0)
            prep_b(1)
            bnsb = res.tile([P, 4 * KT], F32, tag="bnv", name="bnv")
            nc.scalar.dma_start(bnsb[:], bnv_d.ap().rearrange("v (t p) -> p (v t)", p=P))
            x_dma(3)
            x_dma(4)
            prep_b(2)
            prep_b(3)
            x_dma(5)
            x_dma(6)
            prep_b(4)
            prep_b(5)
            x_dma(7)
            w2sb = res.tile([P, KT, NPOS, C], FP8, tag="w2q", name="w2q")
            nc.scalar.dma_start(w2sb[:], w2_d.ap())
            prep_b(6)
            prep_b(7)

            inv1sb = bnsb[:, 0 * KT:1 * KT]
            nb1sb = bnsb[:, 1 * KT:2 * KT]
            inv2sb = bnsb[:, 2 * KT:3 * KT]
            b2sb = bnsb[:, 3 * KT:4 * KT]

            def conv_groups(b, mt, wsb, src):
                """6-tap DoubleRow accumulation for the NJ chunks of (b, mt).

                pos-outer / chunk-inner so consecutive matmuls share lhsT.
                Returns the NJ psum tiles.
                """
                pts = [
                    ps.tile([P, CH], F32, tag="ps", name=f"ps_{b}_{mt}_{j}")
                    for j in range(NJ)
                ]
                for pos in range(NPOS):
                    kh, kw = divmod(pos, 3)
                    off = kh * PITCH + kw
                    for j in range(NJ):
                        nc.tensor.matmul(
                            pts[j][:],
                            wsb[:, :, pos, mt * P:(mt + 1) * P],
                            src[:, :, off + j * CH: off + j * CH + CH],
                            start=(pos == 0),
                            stop=(pos == NPOS - 1),
                            perf_mode=mybir.MatmulPerfMode.DoubleRow,
                        )
                return pts

            # ---- conv1 + binarize epilogue ----
            for b in range(BL):
                for mt in range(KT):
                    pts = conv_groups(b, mt, w1sb, xq1[b])
                    q2v = interior(xq2[b], mt)
                    for j in range(NJ):
                        r = ROWS_J[j]
                        nc.vector.tensor_scalar(
                            q2v[:, 11 * j:11 * j + r, :],
                            pts[j].rearrange("c (r w) -> c r w", w=PITCH)[:, 0:r, 0:W],
                            inv1sb[:, mt:mt + 1],
                            nb1sb[:, mt:mt + 1],
                            mybir.AluOpType.mult,
                            mybir.AluOpType.is_gt,
                        )

            # ---- conv2 + bn2 + residual + relu ----
            OUTQ = (nc.sync, nc.gpsimd, nc.scalar)
            for b in range(BL):
                for mt in range(KT):
                    pts = conv_groups(b, mt, w2sb, xq2[b])
                    ot = stg.tile([P, H * W], F32, tag="ot", name=f"ot_{b}_{mt}")
                    for j in range(NJ):
                        r = ROWS_J[j]
                        n = r * W
                        n0 = 11 * j * W
                        tt = tmp.tile([P, 11 * W], F32, tag="t2", name=f"t2_{b}_{mt}_{j}")
                        nc.vector.scalar_tensor_tensor(
                            tt[:, 0:n].rearrange("c (r w) -> c r w", w=W),
                            pts[j].rearrange("c (r w) -> c r w", w=PITCH)[:, 0:r, 0:W],
                            inv2sb[:, mt:mt + 1],
                            xf(mt, b)[:, n0:n0 + n].rearrange("c (r w) -> c r w", w=W),
                            mybir.AluOpType.mult,
                            mybir.AluOpType.add,
                        )
                        nc.scalar.activation(
                            ot[:, n0:n0 + n], tt[:, 0:n],
                            mybir.ActivationFunctionType.Relu,
                            bias=b2sb[:, mt:mt + 1],
                            scale=1.0,
                        )
                    OUTQ[(b * KT + mt) % 3].dma_start(
                        out_d.ap()[b, mt * P:(mt + 1) * P].rearrange("c h w -> c (h w)"),
                        ot[:],
                    )

    nc.compile()
    _CACHE["nc"] = nc
    return nc


def _prep(w1, w2, gamma1, beta1, mean1, var1, gamma2, beta2, mean2, var2):
    """Host-side: fold BN, binarize + lay out weights as lhsT tiles."""
    def fold(gamma, beta, mean, var):
        inv = (gamma.astype(np.float64) / np.sqrt(var.astype(np.float64) + EPS))
        inv = inv.astype(np.float32)
        bias = (beta.astype(np.float32) - mean.astype(np.float32) * inv)
        return inv, bias

    inv1, bias1 = fold(gamma1, beta1, mean1, var1)
    inv2, bias2 = fold(gamma2, beta2, mean2, var2)

    if VARIANT == "fp8":
        def wt(w):
            # [O, I, 2, 3] -> DoubleRow lhsT layout [ci, ko, pos, co]
            s = np.sign(w).astype(np.float32)
            arr = s.transpose(1, 2, 3, 0).reshape(KT, P, NPOS, C).transpose(1, 0, 2, 3)
            return np.ascontiguousarray(arr).astype(mybir.dt.np(FP8))
    else:
        def wt(w):
            # [O, I, 2, 3] -> lhsT layout [kt, ci, pos, co]
            s = np.sign(w).astype(ml_dtypes.bfloat16)
            return np.ascontiguousarray(
                s.transpose(1, 2, 3, 0).reshape(KT, P, NPOS, C)
            )

    bnv = np.ascontiguousarray(np.stack([inv1, -bias1, inv2, bias2]))
    return wt(w1), wt(w2), bnv


def _in_maps(x, w1t, w2t, bnv):
    """Per-core input dicts; x is transposed to the [p, b, kt, hw] layout."""
    maps = []
    for c in range(N_CORES):
        xs = x[c * BL:(c + 1) * BL]                       # [BL, C, H, W]
        xh = np.ascontiguousarray(
            xs.reshape(BL, KT, P, H * W).transpose(2, 0, 1, 3))
        maps.append({"x": xh, "w1t": w1t, "w2t": w2t, "bnv": bnv})
    return maps


def kernel(x, w1, gamma1, beta1, mean1, var1,
           w2, gamma2, beta2, mean2, var2):
    x = np.asarray(x, dtype=np.float32)
    w1t, w2t, bnv = _prep(
        np.asarray(w1), np.asarray(w2),
        np.asarray(gamma1), np.asarray(beta1), np.asarray(mean1), np.asarray(var1),
        np.asarray(gamma2), np.asarray(beta2), np.asarray(mean2), np.asarray(var2),
    )

    nc = _build()
    in_maps = _in_maps(x, w1t, w2t, bnv)

    res = run_bass_kernel_spmd(nc, in_maps, core_ids=list(range(N_CORES)))
    out = np.concatenate([r["out"] for r in res.results], axis=0)
    return out
